# revision 5
# baseline (speedup 1.0000x reference)
"""DLinear layer (nn_DLinearLayer) TRN2 Bass kernel — single-GEMM formulation.

Math (reference):
    trend[b,t,f]  = avgpool2(x)[b,t,f] = 0.5*(x[t]+x[t+1]), last: x[T-1]
    resid         = x - trend
    out[b,n,f]    = trend[:,:,f] @ trend_W[f] + trend_b[f,n]
                  + resid[:,:,f] @ residual_W[f] + residual_b[f,n]

Identity: with B[t] = x[t+1] (B[T-1] = x[T-1]), trend = (x+B)/2,
resid = (x-B)/2, and shift(x) @ V == x @ V' where V'[s] = V[s-1]
(V'[0] = 0, V'[T-1] += V[T-1]) — the layer folds to ONE GEMM/feature:
    out[:, :, f] = x[:, :, f] @ Weff[f] + (tb+rb)[f]
    Weff[f] = (Wt[f]+Wr[f])/2 + shift_down((Wt[f]-Wr[f])/2) [+ last-row fixup]
Weff precomputed on host; bias added on host. Halves device FLOPs and
weight DMA vs the 2-GEMM form.

Sharding: feature-expert — core k owns features {2k, 2k+1}; every weight
byte is read exactly once across the system.

Perf model (measured via perfetto on this container):
  * PE: 128x128x512 bf16 matmul ~250ns, dense once fed -> 64 matmuls
    ~16us is the hard floor; everything else must hide under it.
  * DMA: 16 channels x ~23GB/s (~410GB/s pool, lines >= 2KB). Each
    dma_start costs ~0.65us sequencer issue + ~2us issue->first-
    descriptor latency per queue; so the FIRST W chunk is the FIRST
    dma_start of its queue, x rides the SWDGE (gpsimd) as a third
    generation stream, and bulk W uses 4KB-line 2-chunk groups.
  * ACT_TABLE_LOAD (~1.3us) fires at the scalar engine's first
    activation op — absorbed mid-run by giving ACT one f0 drain cast,
    keeping it off both the load stream start and the tail.
  * Tail: last stop-matmul -> cast (DVE/ACT in parallel) -> per-psum
    [128,512] stores so nothing waits on a sibling half.
Dtypes: x / Weff / out bf16 (PSUM fp32): rel-l2 ~2.9e-3 (gate 2e-2).
"""

import numpy as np

import concourse.bass as bass
import concourse.mybir as mybir
import concourse.tile as tile
from concourse.bass_utils import run_bass_kernel_spmd

F, B, T, N = 16, 256, 1024, 1024
NCORES = 8
FL = F // NCORES          # features per core
TC = T // 128             # contraction chunks (t on SBUF partitions)
NB = B // 128             # output partition tiles
NH = N // 512             # output free-dim halves (one PSUM bank each)
NG = TC // 2              # W groups (2 chunks = 4KB lines) per feature
F32 = mybir.dt.float32
BF16 = mybir.dt.bfloat16


def _split_multi_waits(nc):
    """This container's walrus build accepts at most ONE sem wait per
    instruction ("Too many sync wait commands" in CoreV3Gen setupSyncWait).
    Tile emits 2+. Move excess waits onto nofuse NoOps placed immediately
    before the owning instruction on the same engine: engines execute their
    stream in order, so semantics are unchanged."""
    for fn in nc.m.functions:
        for blk in fn.blocks:
            out = []
            for inst in blk.instructions:
                si = inst.sync_info
                if si is not None and si.on_wait and len(si.on_wait) > 1:
                    waits = list(si.on_wait)
                    for j, w in enumerate(waits[:-1]):
                        out.append(mybir.InstNoOp(
                            name=f"{inst.name}-ws{j}",
                            engine=inst.engine,
                            bass_nofuse=True,
                            sync_info=mybir.SyncInfo(on_wait=[w], on_update=[]),
                        ))
                    si.on_wait = [waits[-1]]
                out.append(inst)
            blk.instructions[:] = out


def _build():
    nc = bass.Bass(trn_type="TRN2")

    # partition-major layouts: long contiguous per-partition DRAM lines
    #   xP[f, p, c, b] = x[b, c*128+p, f]        (full-x line = 4KB)
    #   wP[f, p, c, n] = Weff[f, c*128+p, n]     (2-chunk group line = 4KB)
    x_d = nc.dram_tensor("xP", [FL, 128, TC, B], BF16, kind="ExternalInput")
    w_d = nc.dram_tensor("wP", [FL, 128, TC, N], BF16, kind="ExternalInput")
    out_d = nc.dram_tensor("out", [FL, B, N], BF16, kind="ExternalOutput")

    with tile.TileContext(nc) as tc:
        with (
            tc.tile_pool(name="xp", bufs=2) as xp,
            tc.tile_pool(name="wp", bufs=FL * NG) as wp,
            tc.tile_pool(name="ob", bufs=FL * NB) as obp,
            tc.tile_pool(name="ps", bufs=8, space="PSUM") as psp,
        ):
            q0, q1 = nc.sync, nc.scalar   # the two HWDGE queues

            xt = {f: xp.tile([128, TC, B], BF16, tag="x", name=f"x{f}")
                  for f in range(FL)}
            wt = {(f, g): wp.tile([128, 2, N], BF16, tag="w", name=f"w{f}_{g}")
                  for f in range(FL) for g in range(NG)}

            def wload(eng, f, g):
                eng.dma_start(wt[f, g][:], w_d[f, :, 2 * g:2 * g + 2, :])

            # x: third generation stream (SWDGE) so HWDGE queue heads are
            # the critical W chunks. x0 first; x1 has ~8us of slack.
            nc.gpsimd.dma_start(xt[0][:], x_d[0, :, :, :])
            nc.gpsimd.dma_start(xt[1][:], x_d[1, :, :, :])

            # W: chunk c0 / c1 are the FIRST dma_start on q1 / q0 (their
            # ~2us first-descriptor latency runs concurrently); the rest
            # stream as 2-chunk groups in consumption order.
            q1.dma_start(wt[0, 0][:, 0, :], w_d[0, :, 0:1, :])
            q0.dma_start(wt[0, 0][:, 1, :], w_d[0, :, 1:2, :])
            wload(q0, 0, 1)      # c2-3
            wload(q1, 0, 2)      # c4-5
            wload(q0, 0, 3)      # c6-7
            wload(q1, 1, 0)
            wload(q0, 1, 1)
            wload(q1, 1, 2)
            wload(q0, 1, 3)

            # ---- GEMM chains: psum[b,h] accumulates over the 8 t-chunks;
            # (c, b, h) order reuses the stationary x[c,b] for both
            # N-halves (halves LDWEIGHTS).
            for f in range(FL):
                ps = {(b, h): psp.tile([128, 512], F32, tag="ps",
                                       name=f"ps{f}_{b}_{h}")
                      for b in range(NB) for h in range(NH)}
                for c in range(TC):
                    g, j = divmod(c, 2)
                    for b in range(NB):
                        for h in range(NH):
                            ns = slice(h * 512, (h + 1) * 512)
                            nc.tensor.matmul(
                                ps[b, h][:],
                                xt[f][:, c, b * 128:(b + 1) * 128],
                                wt[f, g][:, j, ns],
                                start=(c == 0), stop=(c == TC - 1))
                # drain casts fp32 psum -> bf16 sbuf. ACT takes one h-half
                # per b (its first cast in f0 absorbs ACT_TABLE_LOAD well
                # before the tail); DVE takes the other.
                for b in range(NB):
                    ot = obp.tile([128, N], BF16, tag="o", name=f"o{f}_{b}")
                    nc.vector.tensor_copy(ot[:, 0:512], ps[b, 0][:])
                    nc.scalar.copy(ot[:, 512:1024], ps[b, 1][:])
                    bs = slice(b * 128, (b + 1) * 128)
                    if f < FL - 1:
                        # mid-run store, full row, SWDGE (2KB lines)
                        nc.gpsimd.dma_start(out_d[f, bs, :], ot[:])
                    else:
                        # tail: store each half as soon as its cast lands
                        q0.dma_start(out_d[f, bs, 0:512], ot[:, 0:512])
                        q1.dma_start(out_d[f, bs, 512:1024], ot[:, 512:1024])

    _split_multi_waits(nc)
    return nc


_NC_CACHE = []


def kernel(**inputs) -> np.ndarray:
    import ml_dtypes

    x = np.asarray(inputs["history_in"], dtype=np.float32)     # [B, T, F]
    wtr = np.asarray(inputs["trend_W"], dtype=np.float32)      # [F, T, N]
    wre = np.asarray(inputs["residual_W"], dtype=np.float32)   # [F, T, N]
    tb = np.asarray(inputs["trend_b"], dtype=np.float32)       # [F, N]
    rb = np.asarray(inputs["residual_b"], dtype=np.float32)    # [F, N]

    # fold trend+residual GEMMs into one effective weight (fp32 math,
    # single bf16 rounding at the end)
    v = (wtr - wre) * 0.5
    weff = (wtr + wre) * 0.5
    weff[:, 1:, :] += v[:, :-1, :]
    weff[:, T - 1, :] += v[:, T - 1, :]

    # partition-major repacks (see _build docstring)
    xP = np.ascontiguousarray(
        x.transpose(2, 1, 0).reshape(F, TC, 128, B).transpose(0, 2, 1, 3)
    ).astype(ml_dtypes.bfloat16)                               # [F,128,TC,B]
    wP = np.ascontiguousarray(
        weff.reshape(F, TC, 128, N).transpose(0, 2, 1, 3)
    ).astype(ml_dtypes.bfloat16)                               # [F,128,TC,N]

    if not _NC_CACHE:
        _NC_CACHE.append(_build())
    nc = _NC_CACHE[0]

    in_maps = []
    for k in range(NCORES):
        sl = slice(FL * k, FL * (k + 1))
        in_maps.append({
            "xP": np.ascontiguousarray(xP[sl]),
            "wP": np.ascontiguousarray(wP[sl]),
        })

    res = run_bass_kernel_spmd(nc, in_maps, core_ids=list(range(NCORES)))
    full = np.concatenate(
        [np.asarray(r["out"]) for r in res.results], axis=0)   # [F, B, N] bf16
    out = full.astype(np.float32).transpose(1, 2, 0)           # [B, N, F]
    out += (tb + rb).T[None, :, :]                             # host bias
    return np.ascontiguousarray(out)


# revision 7
# speedup vs baseline: 1.0868x; 1.0868x over previous
"""DLinear layer (nn_DLinearLayer) TRN2 Bass kernel — single-GEMM formulation.

Math (reference):
    trend[b,t,f]  = avgpool2(x)[b,t,f] = 0.5*(x[t]+x[t+1]), last: x[T-1]
    resid         = x - trend
    out[b,n,f]    = trend[:,:,f] @ trend_W[f] + trend_b[f,n]
                  + resid[:,:,f] @ residual_W[f] + residual_b[f,n]

Identity: with B[t] = x[t+1] (B[T-1] = x[T-1]), trend = (x+B)/2,
resid = (x-B)/2, and shift(x) @ V == x @ V' where V'[s] = V[s-1]
(V'[0] = 0, V'[T-1] += V[T-1]) — the layer folds to ONE GEMM/feature:
    out[:, :, f] = x[:, :, f] @ Weff[f] + (tb+rb)[f]
    Weff[f] = (Wt[f]+Wr[f])/2 + shift_down((Wt[f]-Wr[f])/2) [+ last-row fixup]
Weff precomputed on host; bias added on host. Halves device FLOPs and
weight DMA vs the 2-GEMM form.

Sharding: feature-expert — core k owns features {2k, 2k+1}; every weight
byte is read exactly once across the system.

Perf model (measured via perfetto on this container):
  * PE: 128x128x512 bf16 matmul = 216ns when fed (1 col/cycle); 64
    matmuls ~= 14-16us is the hard floor. Keep it dense from the
    earliest possible start.
  * DMA: 16 channels x ~23GB/s (~410GB/s pool, lines >= 2KB; the
    packed host layouts below give 2-4KB lines). Each dma_start costs
    ~0.65us issue + ~2us to first descriptor, per queue, so W chunks
    alternate strictly between the two HWDGE queues in consumption
    order (per-queue ~205GB/s x 2 beats the PE's 239GB/s burn) and the
    critical c0 chunk is q1's FIRST dma_start while q0 leads with x.
  * SWDGE (gpsimd) descriptors starve (~100GB/s) while HWDGE saturates
    the pool -> only the mid-run f0 stores ride it (idle window).
  * ACT_TABLE_LOAD (1.3us) fires lazily before the scalar engine's
    first activation op and can stall q1's issue stream -> all drain
    casts go on DVE (tensor_copy needs no table).
  * Tail: the last feature's b1 chains stop two matmuls early (chunk
    order swapped) so their casts+stores hide under b0's final matmuls.
Dtypes: x / Weff / out bf16 (PSUM fp32): rel-l2 ~2.9e-3 (gate 2e-2).
"""

import numpy as np

import concourse.bass as bass
import concourse.mybir as mybir
import concourse.tile as tile
from concourse.bass_utils import run_bass_kernel_spmd

F, B, T, N = 16, 256, 1024, 1024
NCORES = 8
FL = F // NCORES          # features per core
TC = T // 128             # contraction chunks (t on SBUF partitions)
NB = B // 128             # output partition tiles
NH = N // 512             # output free-dim halves (one PSUM bank each)
HALF = TC // 2
F32 = mybir.dt.float32
BF16 = mybir.dt.bfloat16


def _split_multi_waits(nc):
    """This container's walrus build accepts at most ONE sem wait per
    instruction ("Too many sync wait commands" in CoreV3Gen setupSyncWait).
    Tile emits 2+. Move excess waits onto nofuse NoOps placed immediately
    before the owning instruction on the same engine: engines execute their
    stream in order, so semantics are unchanged."""
    for fn in nc.m.functions:
        for blk in fn.blocks:
            out = []
            for inst in blk.instructions:
                si = inst.sync_info
                if si is not None and si.on_wait and len(si.on_wait) > 1:
                    waits = list(si.on_wait)
                    for j, w in enumerate(waits[:-1]):
                        out.append(mybir.InstNoOp(
                            name=f"{inst.name}-ws{j}",
                            engine=inst.engine,
                            bass_nofuse=True,
                            sync_info=mybir.SyncInfo(on_wait=[w], on_update=[]),
                        ))
                    si.on_wait = [waits[-1]]
                out.append(inst)
            blk.instructions[:] = out


def _build():
    nc = bass.Bass(trn_type="TRN2")

    # partition-major layouts: long contiguous per-partition DRAM lines
    #   xP[f, p, c, b] = x[b, c*128+p, f]     (half-x line = 2KB)
    #   wP[f, p, c, n] = Weff[f, c*128+p, n]  (chunk line = 2KB)
    x_d = nc.dram_tensor("xP", [FL, 128, TC, B], BF16, kind="ExternalInput")
    w_d = nc.dram_tensor("wP", [FL, 128, TC, N], BF16, kind="ExternalInput")
    out_d = nc.dram_tensor("out", [FL, B, N], BF16, kind="ExternalOutput")

    with tile.TileContext(nc) as tc:
        with (
            tc.tile_pool(name="xp", bufs=2) as xp,
            tc.tile_pool(name="wp", bufs=FL * TC) as wp,
            tc.tile_pool(name="ob", bufs=FL * NB) as obp,
            tc.tile_pool(name="ps", bufs=8, space="PSUM") as psp,
        ):
            q0, q1 = nc.sync, nc.scalar   # the two HWDGE queues

            xt = {f: xp.tile([128, TC, B], BF16, tag="x", name=f"x{f}")
                  for f in range(FL)}
            wt = {(f, c): wp.tile([128, N], BF16, tag="w", name=f"w{f}_{c}")
                  for f in range(FL) for c in range(TC)}

            def wload(eng, f, c):
                eng.dma_start(wt[f, c][:], w_d[f, :, c, :])

            # loads, strictly alternating queues in consumption order;
            # q1 leads with the gating W chunk c0, q0 with the x half it
            # needs. x second halves / x1 ride the same alternation at
            # the point they're needed.
            q0.dma_start(xt[0][:, 0:HALF, :], x_d[0, :, 0:HALF, :])
            wload(q1, 0, 0)
            wload(q0, 0, 1)
            q1.dma_start(xt[0][:, HALF:TC, :], x_d[0, :, HALF:TC, :])
            wload(q1, 0, 2)
            wload(q0, 0, 3)
            wload(q1, 0, 4)
            wload(q0, 0, 5)
            wload(q1, 0, 6)
            wload(q0, 0, 7)
            q0.dma_start(xt[1][:, 0:HALF, :], x_d[1, :, 0:HALF, :])
            q1.dma_start(xt[1][:, HALF:TC, :], x_d[1, :, HALF:TC, :])
            wload(q1, 1, 0)
            wload(q0, 1, 1)
            wload(q1, 1, 2)
            wload(q0, 1, 3)
            wload(q1, 1, 4)
            wload(q0, 1, 5)
            wload(q1, 1, 6)
            wload(q0, 1, 7)

            # ---- GEMM chains: psum[b,h] accumulates over the 8 t-chunks,
            # (c, b, h) order. For the LAST feature the final two chunks
            # are emitted b1-first so b1's chains stop ~0.5us before b0's
            # and their drain hides under b0's final matmuls.
            def mm(f, ps, c, b, h, start, stop):
                ns = slice(h * 512, (h + 1) * 512)
                nc.tensor.matmul(
                    ps[b, h][:],
                    xt[f][:, c, b * 128:(b + 1) * 128],
                    wt[f, c][:, ns],
                    start=start, stop=stop)

            casts = {}   # (f,b,h) -> (ot, stop order index)
            for f in range(FL):
                ps = {(b, h): psp.tile([128, 512], F32, tag="ps",
                                       name=f"ps{f}_{b}_{h}")
                      for b in range(NB) for h in range(NH)}
                last = FL - 1
                ncut = TC - 2 if f == last else TC
                for c in range(ncut):
                    for b in range(NB):
                        for h in range(NH):
                            mm(f, ps, c, b, h, c == 0, c == TC - 1)
                if f == last:
                    for b in (1, 0):
                        for c in (TC - 2, TC - 1):
                            for h in range(NH):
                                mm(f, ps, c, b, h, False, c == TC - 1)
                # drain casts fp32 psum -> bf16, all on DVE (no ACT table)
                ots = {b: obp.tile([128, N], BF16, tag="o", name=f"o{f}_{b}")
                       for b in range(NB)}
                border = (1, 0) if f == last else (0, 1)
                for b in border:
                    for h in range(NH):
                        ns = slice(h * 512, (h + 1) * 512)
                        nc.vector.tensor_copy(ots[b][:, ns], ps[b, h][:])
                # stores: f0 full rows on SWDGE (idle mid-run window);
                # last feature per-(b,h) halves on the HWDGE queues, b1
                # first, so each store chases its own cast.
                for b in border:
                    bs = slice(b * 128, (b + 1) * 128)
                    if f < last:
                        nc.gpsimd.dma_start(out_d[f, bs, :], ots[b][:])
                    else:
                        q0.dma_start(out_d[f, bs, 0:512], ots[b][:, 0:512])
                        q1.dma_start(out_d[f, bs, 512:1024],
                                     ots[b][:, 512:1024])

    _split_multi_waits(nc)
    return nc


_NC_CACHE = []


def kernel(**inputs) -> np.ndarray:
    import ml_dtypes

    x = np.asarray(inputs["history_in"], dtype=np.float32)     # [B, T, F]
    wtr = np.asarray(inputs["trend_W"], dtype=np.float32)      # [F, T, N]
    wre = np.asarray(inputs["residual_W"], dtype=np.float32)   # [F, T, N]
    tb = np.asarray(inputs["trend_b"], dtype=np.float32)       # [F, N]
    rb = np.asarray(inputs["residual_b"], dtype=np.float32)    # [F, N]

    # fold trend+residual GEMMs into one effective weight (fp32 math,
    # single bf16 rounding at the end)
    v = (wtr - wre) * 0.5
    weff = (wtr + wre) * 0.5
    weff[:, 1:, :] += v[:, :-1, :]
    weff[:, T - 1, :] += v[:, T - 1, :]

    # partition-major repacks (see _build docstring)
    xP = np.ascontiguousarray(
        x.transpose(2, 1, 0).reshape(F, TC, 128, B).transpose(0, 2, 1, 3)
    ).astype(ml_dtypes.bfloat16)                               # [F,128,TC,B]
    wP = np.ascontiguousarray(
        weff.reshape(F, TC, 128, N).transpose(0, 2, 1, 3)
    ).astype(ml_dtypes.bfloat16)                               # [F,128,TC,N]

    if not _NC_CACHE:
        _NC_CACHE.append(_build())
    nc = _NC_CACHE[0]

    in_maps = []
    for k in range(NCORES):
        sl = slice(FL * k, FL * (k + 1))
        in_maps.append({
            "xP": np.ascontiguousarray(xP[sl]),
            "wP": np.ascontiguousarray(wP[sl]),
        })

    res = run_bass_kernel_spmd(nc, in_maps, core_ids=list(range(NCORES)))
    full = np.concatenate(
        [np.asarray(r["out"]) for r in res.results], axis=0)   # [F, B, N] bf16
    out = full.astype(np.float32).transpose(1, 2, 0)           # [B, N, F]
    out += (tb + rb).T[None, :, :]                             # host bias
    return np.ascontiguousarray(out)


# revision 15
# speedup vs baseline: 1.0871x; 1.0002x over previous
"""DLinear layer (nn_DLinearLayer) TRN2 Bass kernel — single-GEMM formulation.

Math (reference):
    trend[b,t,f]  = avgpool2(x)[b,t,f] = 0.5*(x[t]+x[t+1]), last: x[T-1]
    resid         = x - trend
    out[b,n,f]    = trend[:,:,f] @ trend_W[f] + trend_b[f,n]
                  + resid[:,:,f] @ residual_W[f] + residual_b[f,n]

Identity: with B[t] = x[t+1] (B[T-1] = x[T-1]), trend = (x+B)/2,
resid = (x-B)/2, and shift(x) @ V == x @ V' where V'[s] = V[s-1]
(V'[0] = 0, V'[T-1] += V[T-1]) — the layer folds to ONE GEMM/feature:
    out[:, :, f] = x[:, :, f] @ Weff[f] + (tb+rb)[f]
    Weff[f] = (Wt[f]+Wr[f])/2 + shift_down((Wt[f]-Wr[f])/2) [+ last-row fixup]
Weff precomputed on host; bias added on host. Halves device FLOPs and
weight DMA vs the 2-GEMM form.

Sharding: feature-expert — core k owns features {2k, 2k+1}; every weight
byte is read exactly once across the system.

Perf model (measured via perfetto on this container):
  * PE: 128x128x512 bf16 matmul = 216ns when fed (1 col/cycle); 64
    matmuls ~= 14-16us is the hard floor. Keep it dense from the
    earliest possible start.
  * DMA: 16 channels x ~23GB/s (~410GB/s pool, lines >= 2KB; the
    packed host layouts below give 2-4KB lines). Each dma_start costs
    ~0.65us issue + ~2us to first descriptor, per queue, so W chunks
    alternate strictly between the two HWDGE queues in consumption
    order (per-queue ~205GB/s x 2 beats the PE's 239GB/s burn) and the
    critical c0 chunk is q1's FIRST dma_start while q0 leads with x.
  * SWDGE (gpsimd) descriptors starve (~100GB/s) while HWDGE saturates
    the pool -> only the mid-run f0 stores ride it (idle window).
  * ACT_TABLE_LOAD (1.3us) fires lazily before the scalar engine's
    first activation op and can stall q1's issue stream -> all drain
    casts go on DVE (tensor_copy needs no table).
  * Tail: the last feature's b1 chains stop two matmuls early (chunk
    order swapped) so their casts+stores hide under b0's final matmuls.
Dtypes: x / Weff / out bf16 (PSUM fp32): rel-l2 ~2.9e-3 (gate 2e-2).
"""

import numpy as np

import concourse.bass as bass
import concourse.mybir as mybir
import concourse.tile as tile
from concourse.bass_utils import run_bass_kernel_spmd

F, B, T, N = 16, 256, 1024, 1024
NCORES = 8
FL = F // NCORES          # features per core
TC = T // 128             # contraction chunks (t on SBUF partitions)
NB = B // 128             # output partition tiles
NH = N // 512             # output free-dim halves (one PSUM bank each)
HALF = TC // 2
F32 = mybir.dt.float32
BF16 = mybir.dt.bfloat16


def _split_multi_waits(nc):
    """This container's walrus build accepts at most ONE sem wait per
    instruction ("Too many sync wait commands" in CoreV3Gen setupSyncWait).
    Tile emits 2+. Move excess waits onto nofuse NoOps placed immediately
    before the owning instruction on the same engine: engines execute their
    stream in order, so semantics are unchanged."""
    for fn in nc.m.functions:
        for blk in fn.blocks:
            out = []
            for inst in blk.instructions:
                si = inst.sync_info
                if si is not None and si.on_wait and len(si.on_wait) > 1:
                    waits = list(si.on_wait)
                    for j, w in enumerate(waits[:-1]):
                        out.append(mybir.InstNoOp(
                            name=f"{inst.name}-ws{j}",
                            engine=inst.engine,
                            bass_nofuse=True,
                            sync_info=mybir.SyncInfo(on_wait=[w], on_update=[]),
                        ))
                    si.on_wait = [waits[-1]]
                out.append(inst)
            blk.instructions[:] = out


def _build():
    nc = bass.Bass(trn_type="TRN2")

    # partition-major layouts: long contiguous per-partition DRAM lines
    #   xP[f, p, c, b] = x[b, c*128+p, f]     (half-x line = 2KB)
    #   wP[f, p, c, n] = Weff[f, c*128+p, n]  (chunk line = 2KB)
    x_d = nc.dram_tensor("xP", [FL, 128, TC, B], BF16, kind="ExternalInput")
    w_d = nc.dram_tensor("wP", [FL, 128, TC, N], BF16, kind="ExternalInput")
    out_d = nc.dram_tensor("out", [FL, B, N], BF16, kind="ExternalOutput")

    with tile.TileContext(nc) as tc:
        with (
            tc.tile_pool(name="xp", bufs=2) as xp,
            tc.tile_pool(name="wp", bufs=FL * TC) as wp,
            tc.tile_pool(name="ob", bufs=FL * NB) as obp,
            tc.tile_pool(name="ps", bufs=8, space="PSUM") as psp,
        ):
            q0, q1 = nc.sync, nc.scalar   # the two HWDGE queues

            xt = {f: xp.tile([128, TC, B], BF16, tag="x", name=f"x{f}")
                  for f in range(FL)}
            wt = {(f, c): wp.tile([128, N], BF16, tag="w", name=f"w{f}_{c}")
                  for f in range(FL) for c in range(TC)}

            def wload(eng, f, c):
                eng.dma_start(wt[f, c][:], w_d[f, :, c, :])

            # loads. The DGE round-robins descriptors across all in-
            # flight dma_starts, so a transfer's completion time scales
            # with the whole outstanding window — the PE-gating pieces
            # (W c0 halves, x chunks 0-1) are therefore SMALL (128KB),
            # and everything else is ordered by when the PE needs it.
            def xload(eng, f, c0, c1):
                eng.dma_start(xt[f][:, c0:c1, :], x_d[f, :, c0:c1, :])

            xload(q0, 0, 0, 2)                    # x chunks 0-1 (128KB)
            q1.dma_start(wt[0, 0][:, 0:512], w_d[0, :, 0, 0:512])
            xload(q0, 0, 2, 4)
            q1.dma_start(wt[0, 0][:, 512:1024], w_d[0, :, 0, 512:1024])
            wload(q0, 0, 1)
            wload(q1, 0, 2)
            wload(q0, 0, 3)
            xload(q1, 0, 4, 8)                    # x chunks 4-7
            wload(q0, 0, 5)
            wload(q1, 0, 4)
            wload(q0, 0, 7)
            wload(q1, 0, 6)
            xload(q0, 1, 0, 4)
            xload(q1, 1, 4, 8)
            wload(q0, 1, 1)
            wload(q1, 1, 0)
            wload(q0, 1, 3)
            wload(q1, 1, 2)
            wload(q0, 1, 5)
            wload(q1, 1, 4)
            wload(q0, 1, 7)
            wload(q1, 1, 6)

            # ---- GEMM chains: psum[b,h] accumulates over the 8 t-chunks,
            # (c, b, h) order. For the LAST feature the final two chunks
            # are emitted b1-first so b1's chains stop ~0.5us before b0's
            # and their drain hides under b0's final matmuls.
            def mm(f, ps, c, b, h, start, stop):
                ns = slice(h * 512, (h + 1) * 512)
                nc.tensor.matmul(
                    ps[b, h][:],
                    xt[f][:, c, b * 128:(b + 1) * 128],
                    wt[f, c][:, ns],
                    start=start, stop=stop)

            casts = {}   # (f,b,h) -> (ot, stop order index)
            for f in range(FL):
                ps = {(b, h): psp.tile([128, 512], F32, tag="ps",
                                       name=f"ps{f}_{b}_{h}")
                      for b in range(NB) for h in range(NH)}
                last = FL - 1
                ncut = TC - 2 if f == last else TC
                for c in range(ncut):
                    for b in range(NB):
                        for h in range(NH):
                            mm(f, ps, c, b, h, c == 0, c == TC - 1)
                if f == last:
                    for b in (1, 0):
                        for c in (TC - 2, TC - 1):
                            for h in range(NH):
                                mm(f, ps, c, b, h, False, c == TC - 1)
                # drain casts fp32 psum -> bf16, all on DVE (DMA from PSUM
                # and gpsimd-PSUM are both illegal; ACT would pay a 1.3us
                # ACT_TABLE_LOAD in the load window). b1-early stop order
                # lets the last feature's b1 drains hide under b0's final
                # matmuls; each store chases its own cast.
                ots = {b: obp.tile([128, N], BF16, tag="o", name=f"o{f}_{b}")
                       for b in range(NB)}
                border = (1, 0) if f == last else (0, 1)
                for b in border:
                    bs = slice(b * 128, (b + 1) * 128)
                    if f < last:
                        for h in range(NH):
                            ns = slice(h * 512, (h + 1) * 512)
                            nc.vector.tensor_copy(ots[b][:, ns], ps[b, h][:])
                        nc.gpsimd.dma_start(out_d[f, bs, :], ots[b][:])
                    else:
                        nc.vector.tensor_copy(ots[b][:, 0:512], ps[b, 0][:])
                        q0.dma_start(out_d[f, bs, 0:512], ots[b][:, 0:512])
                        nc.vector.tensor_copy(ots[b][:, 512:1024], ps[b, 1][:])
                        q1.dma_start(out_d[f, bs, 512:1024],
                                     ots[b][:, 512:1024])

    _split_multi_waits(nc)
    return nc


_NC_CACHE = []


def kernel(**inputs) -> np.ndarray:
    import ml_dtypes

    x = np.asarray(inputs["history_in"], dtype=np.float32)     # [B, T, F]
    wtr = np.asarray(inputs["trend_W"], dtype=np.float32)      # [F, T, N]
    wre = np.asarray(inputs["residual_W"], dtype=np.float32)   # [F, T, N]
    tb = np.asarray(inputs["trend_b"], dtype=np.float32)       # [F, N]
    rb = np.asarray(inputs["residual_b"], dtype=np.float32)    # [F, N]

    # fold trend+residual GEMMs into one effective weight (fp32 math,
    # single bf16 rounding at the end)
    v = (wtr - wre) * 0.5
    weff = (wtr + wre) * 0.5
    weff[:, 1:, :] += v[:, :-1, :]
    weff[:, T - 1, :] += v[:, T - 1, :]

    # partition-major repacks (see _build docstring)
    xP = np.ascontiguousarray(
        x.transpose(2, 1, 0).reshape(F, TC, 128, B).transpose(0, 2, 1, 3)
    ).astype(ml_dtypes.bfloat16)                               # [F,128,TC,B]
    wP = np.ascontiguousarray(
        weff.reshape(F, TC, 128, N).transpose(0, 2, 1, 3)
    ).astype(ml_dtypes.bfloat16)                               # [F,128,TC,N]

    if not _NC_CACHE:
        _NC_CACHE.append(_build())
    nc = _NC_CACHE[0]

    in_maps = []
    for k in range(NCORES):
        sl = slice(FL * k, FL * (k + 1))
        in_maps.append({
            "xP": np.ascontiguousarray(xP[sl]),
            "wP": np.ascontiguousarray(wP[sl]),
        })

    res = run_bass_kernel_spmd(nc, in_maps, core_ids=list(range(NCORES)))
    full = np.concatenate(
        [np.asarray(r["out"]) for r in res.results], axis=0)   # [F, B, N] bf16
    out = full.astype(np.float32).transpose(1, 2, 0)           # [B, N, F]
    out = out + (tb + rb).T[None, :, :]                        # host bias
    return np.ascontiguousarray(out)


# revision 16
# speedup vs baseline: 1.0874x; 1.0003x over previous
"""DLinear layer (nn_DLinearLayer) TRN2 Bass kernel — single-GEMM formulation.

Math (reference):
    trend[b,t,f]  = avgpool2(x)[b,t,f] = 0.5*(x[t]+x[t+1]), last: x[T-1]
    resid         = x - trend
    out[b,n,f]    = trend[:,:,f] @ trend_W[f] + trend_b[f,n]
                  + resid[:,:,f] @ residual_W[f] + residual_b[f,n]

Identity: with B[t] = x[t+1] (B[T-1] = x[T-1]), trend = (x+B)/2,
resid = (x-B)/2, and shift(x) @ V == x @ V' where V'[s] = V[s-1]
(V'[0] = 0, V'[T-1] += V[T-1]) — the layer folds to ONE GEMM/feature:
    out[:, :, f] = x[:, :, f] @ Weff[f] + (tb+rb)[f]
    Weff[f] = (Wt[f]+Wr[f])/2 + shift_down((Wt[f]-Wr[f])/2) [+ last-row fixup]
Weff precomputed on host; bias added on host. Halves device FLOPs and
weight DMA vs the 2-GEMM form.

Sharding: feature-expert — core k owns features {2k, 2k+1}; every weight
byte is read exactly once across the system.

Perf model (measured via perfetto on this container):
  * PE: 128x128x512 bf16 matmul = 216ns when fed (1 col/cycle); 64
    matmuls ~= 14-16us is the hard floor. Keep it dense from the
    earliest possible start.
  * DMA: 16 channels x ~23GB/s (~410GB/s pool, lines >= 2KB; the
    packed host layouts below give 2-4KB lines). Each dma_start costs
    ~0.65us issue + ~2us to first descriptor, per queue, so W chunks
    alternate strictly between the two HWDGE queues in consumption
    order (per-queue ~205GB/s x 2 beats the PE's 239GB/s burn) and the
    critical c0 chunk is q1's FIRST dma_start while q0 leads with x.
  * SWDGE (gpsimd) descriptors starve (~100GB/s) while HWDGE saturates
    the pool -> only the mid-run f0 stores ride it (idle window).
  * ACT_TABLE_LOAD (1.3us) fires lazily before the scalar engine's
    first activation op and can stall q1's issue stream -> all drain
    casts go on DVE (tensor_copy needs no table).
  * Tail: the last feature's b1 chains stop two matmuls early (chunk
    order swapped) so their casts+stores hide under b0's final matmuls.
Dtypes: x / Weff / out bf16 (PSUM fp32): rel-l2 ~2.9e-3 (gate 2e-2).
"""

import numpy as np

import concourse.bass as bass
import concourse.mybir as mybir
import concourse.tile as tile
from concourse.bass_utils import run_bass_kernel_spmd

F, B, T, N = 16, 256, 1024, 1024
NCORES = 8
FL = F // NCORES          # features per core
TC = T // 128             # contraction chunks (t on SBUF partitions)
NB = B // 128             # output partition tiles
NH = N // 512             # output free-dim halves (one PSUM bank each)
HALF = TC // 2
F32 = mybir.dt.float32
BF16 = mybir.dt.bfloat16


def _split_multi_waits(nc):
    """This container's walrus build accepts at most ONE sem wait per
    instruction ("Too many sync wait commands" in CoreV3Gen setupSyncWait).
    Tile emits 2+. Move excess waits onto nofuse NoOps placed immediately
    before the owning instruction on the same engine: engines execute their
    stream in order, so semantics are unchanged."""
    for fn in nc.m.functions:
        for blk in fn.blocks:
            out = []
            for inst in blk.instructions:
                si = inst.sync_info
                if si is not None and si.on_wait and len(si.on_wait) > 1:
                    waits = list(si.on_wait)
                    for j, w in enumerate(waits[:-1]):
                        out.append(mybir.InstNoOp(
                            name=f"{inst.name}-ws{j}",
                            engine=inst.engine,
                            bass_nofuse=True,
                            sync_info=mybir.SyncInfo(on_wait=[w], on_update=[]),
                        ))
                    si.on_wait = [waits[-1]]
                out.append(inst)
            blk.instructions[:] = out


def _build():
    nc = bass.Bass(trn_type="TRN2")

    # partition-major layouts: long contiguous per-partition DRAM lines
    #   xP[f, p, c, b] = x[b, c*128+p, f]     (half-x line = 2KB)
    #   wP[f, p, c, n] = Weff[f, c*128+p, n]  (chunk line = 2KB)
    x_d = nc.dram_tensor("xP", [FL, 128, TC, B], BF16, kind="ExternalInput")
    w_d = nc.dram_tensor("wP", [FL, 128, TC, N], BF16, kind="ExternalInput")
    out_d = nc.dram_tensor("out", [FL, B, N], BF16, kind="ExternalOutput")

    with tile.TileContext(nc) as tc:
        with (
            tc.tile_pool(name="xp", bufs=2) as xp,
            tc.tile_pool(name="wp", bufs=FL * TC) as wp,
            tc.tile_pool(name="ob", bufs=FL * NB) as obp,
            tc.tile_pool(name="ps", bufs=8, space="PSUM") as psp,
        ):
            q0, q1 = nc.sync, nc.scalar   # the two HWDGE queues

            xt = {f: xp.tile([128, TC, B], BF16, tag="x", name=f"x{f}")
                  for f in range(FL)}
            wt = {(f, c): wp.tile([128, N], BF16, tag="w", name=f"w{f}_{c}")
                  for f in range(FL) for c in range(TC)}

            def wload(eng, f, c):
                eng.dma_start(wt[f, c][:], w_d[f, :, c, :])

            # loads. DGE empirics (3 traces): each HWDGE queue holds ~4
            # dma_starts in flight; a transfer's completion SEMAPHORE
            # fires around the queue's ring turnover (≈ the 5th issue),
            # then acks stream at ~1 per 1.25us per queue. So the first
            # credit window of each queue carries exactly the chunks the
            # PE burns first (c0..c6 + x0a), x0b/x1 slot in where the
            # ack schedule meets PE need times, and nothing is split
            # smaller than a 256KB chunk (small pieces don't ack
            # earlier, they just waste window slots).
            def xload(eng, f, c0, c1):
                eng.dma_start(xt[f][:, c0:c1, :], x_d[f, :, c0:c1, :])

            xload(q0, 0, 0, HALF)     # q0#1: x chunks 0-3
            wload(q1, 0, 0)           # q1#1: the PE-gating W chunk
            wload(q0, 0, 1)
            wload(q1, 0, 2)
            wload(q0, 0, 3)
            wload(q1, 0, 4)
            xload(q0, 0, HALF, TC)    # q0#4: x chunks 4-7 (need ~c4 time)
            wload(q1, 0, 6)
            wload(q0, 0, 5)
            xload(q1, 1, HALF, TC)
            wload(q0, 0, 7)
            wload(q1, 1, 0)
            xload(q0, 1, 0, HALF)
            wload(q1, 1, 2)
            wload(q0, 1, 1)
            wload(q1, 1, 4)
            wload(q0, 1, 3)
            wload(q1, 1, 6)
            wload(q0, 1, 5)
            wload(q0, 1, 7)

            # ---- GEMM chains: psum[b,h] accumulates over the 8 t-chunks,
            # (c, b, h) order. For the LAST feature the final two chunks
            # are emitted b1-first so b1's chains stop ~0.5us before b0's
            # and their drain hides under b0's final matmuls.
            def mm(f, ps, c, b, h, start, stop):
                ns = slice(h * 512, (h + 1) * 512)
                nc.tensor.matmul(
                    ps[b, h][:],
                    xt[f][:, c, b * 128:(b + 1) * 128],
                    wt[f, c][:, ns],
                    start=start, stop=stop)

            casts = {}   # (f,b,h) -> (ot, stop order index)
            for f in range(FL):
                ps = {(b, h): psp.tile([128, 512], F32, tag="ps",
                                       name=f"ps{f}_{b}_{h}")
                      for b in range(NB) for h in range(NH)}
                last = FL - 1
                ncut = TC - 2 if f == last else TC
                for c in range(ncut):
                    for b in range(NB):
                        for h in range(NH):
                            mm(f, ps, c, b, h, c == 0, c == TC - 1)
                if f == last:
                    for b in (1, 0):
                        for c in (TC - 2, TC - 1):
                            for h in range(NH):
                                mm(f, ps, c, b, h, False, c == TC - 1)
                # drain casts fp32 psum -> bf16, all on DVE (DMA from PSUM
                # and gpsimd-PSUM are both illegal; ACT would pay a 1.3us
                # ACT_TABLE_LOAD in the load window). b1-early stop order
                # lets the last feature's b1 drains hide under b0's final
                # matmuls; each store chases its own cast.
                ots = {b: obp.tile([128, N], BF16, tag="o", name=f"o{f}_{b}")
                       for b in range(NB)}
                border = (1, 0) if f == last else (0, 1)
                for b in border:
                    bs = slice(b * 128, (b + 1) * 128)
                    if f < last:
                        for h in range(NH):
                            ns = slice(h * 512, (h + 1) * 512)
                            nc.vector.tensor_copy(ots[b][:, ns], ps[b, h][:])
                        nc.gpsimd.dma_start(out_d[f, bs, :], ots[b][:])
                    else:
                        nc.vector.tensor_copy(ots[b][:, 0:512], ps[b, 0][:])
                        q0.dma_start(out_d[f, bs, 0:512], ots[b][:, 0:512])
                        nc.vector.tensor_copy(ots[b][:, 512:1024], ps[b, 1][:])
                        q1.dma_start(out_d[f, bs, 512:1024],
                                     ots[b][:, 512:1024])

    _split_multi_waits(nc)
    return nc


_NC_CACHE = []


def kernel(**inputs) -> np.ndarray:
    import ml_dtypes

    x = np.asarray(inputs["history_in"], dtype=np.float32)     # [B, T, F]
    wtr = np.asarray(inputs["trend_W"], dtype=np.float32)      # [F, T, N]
    wre = np.asarray(inputs["residual_W"], dtype=np.float32)   # [F, T, N]
    tb = np.asarray(inputs["trend_b"], dtype=np.float32)       # [F, N]
    rb = np.asarray(inputs["residual_b"], dtype=np.float32)    # [F, N]

    # fold trend+residual GEMMs into one effective weight (fp32 math,
    # single bf16 rounding at the end)
    v = (wtr - wre) * 0.5
    weff = (wtr + wre) * 0.5
    weff[:, 1:, :] += v[:, :-1, :]
    weff[:, T - 1, :] += v[:, T - 1, :]

    # partition-major repacks (see _build docstring)
    xP = np.ascontiguousarray(
        x.transpose(2, 1, 0).reshape(F, TC, 128, B).transpose(0, 2, 1, 3)
    ).astype(ml_dtypes.bfloat16)                               # [F,128,TC,B]
    wP = np.ascontiguousarray(
        weff.reshape(F, TC, 128, N).transpose(0, 2, 1, 3)
    ).astype(ml_dtypes.bfloat16)                               # [F,128,TC,N]

    if not _NC_CACHE:
        _NC_CACHE.append(_build())
    nc = _NC_CACHE[0]

    in_maps = []
    for k in range(NCORES):
        sl = slice(FL * k, FL * (k + 1))
        in_maps.append({
            "xP": np.ascontiguousarray(xP[sl]),
            "wP": np.ascontiguousarray(wP[sl]),
        })

    res = run_bass_kernel_spmd(nc, in_maps, core_ids=list(range(NCORES)))
    full = np.concatenate(
        [np.asarray(r["out"]) for r in res.results], axis=0)   # [F, B, N] bf16
    out = full.astype(np.float32).transpose(1, 2, 0)           # [B, N, F]
    out = out + (tb + rb).T[None, :, :]                        # host bias
    return np.ascontiguousarray(out)
